# revision 9
# baseline (speedup 1.0000x reference)
"""Distributed Bass kernel for nn_AttentionLayer (2-branch GAT-style layer).

Row-shard over 8 NeuronCores (512 rows each). All per-row tensors kept in
"transposed" layout on chip (k on SBUF partitions, own-row i on free) so the
masked softmax feeds the PE attention matmuls without transposes:

  e_b^T[k, i] = prelu(s1_b[i] + s2_b[k])          (one ACT op, alpha=0.2)
  p = exp(e)  (bf16) ;  pt = p * mask01           (multiplicative masking)
  [acc_b; den_b] = [Wh_b | 1]^T @ pt              (fused numerator+denominator)

adj2^T is computed on PE in fp8 DoubleRow (exact: adj is 0/1, psum f32) from a
REPLICATED full-adj input laid out host-side in DoubleRow tile order -- no
AllGather, no collective barrier on the critical path (a dummy AllReduce at
t=0 absorbs the one-time cc-stream init). adj2's diagonal is zeroed by fusing
a per-core (1 - diag) fp8 mask into the min(cnt,1) STT. BatchNorm batch stats
via one tiny AllReduce at the end.
"""

import sys
import numpy as np

for _p in ("/opt/trn_rl_repo", "/opt/trn_rl_repo/concourse"):
    if _p not in sys.path:
        sys.path.insert(0, _p)

import ml_dtypes

N = 4096
M_CORES = 8
R = N // M_CORES          # 512 rows per core
IN_F = 512
HALF = IN_F // 2          # 256
F = 64
P = 128                   # partitions
NT = N // P               # 32 column (k) tiles
NG = 8                    # adj2 psum groups (4 kt each)
G = NT // NG              # 4 kt tiles per group
TP = 8                    # t-pair passes per group load (16 t passes = 8 pairs)
ALPHA = 0.2
EPS = 1e-5
INV_N = 1.0 / N

_CACHED = {}


def build_nc():
    from concourse import bacc, tile, mybir

    f32 = mybir.dt.float32
    bf16 = mybir.dt.bfloat16
    fp8 = mybir.dt.float8e4
    Alu = mybir.AluOpType
    Act = mybir.ActivationFunctionType
    DR = mybir.MatmulPerfMode.DoubleRow

    nc = bacc.Bacc("TRN2", target_bir_lowering=False, debug=False,
                   num_devices=M_CORES)

    hTs_p = nc.declare_dram_parameter("hTs", [P, 4, R], f32, isOutput=False)
    hTfb_p = nc.declare_dram_parameter("hTfb", [P, 4, N], bf16, isOutput=False)
    adjT_p = nc.declare_dram_parameter("adjT", [P, 16, 2, R], fp8,
                                       isOutput=False)
    adjDR_p = nc.declare_dram_parameter("adjDR", [NG, TP, P, 2, 2, R], fp8,
                                        isOutput=False)
    dinv_p = nc.declare_dram_parameter("dinv", [P, NT, R], fp8, isOutput=False)
    Wsb_p = nc.declare_dram_parameter("Wsb", [P, 2, 2 * F], f32,
                                      isOutput=False)
    a1c_p = nc.declare_dram_parameter("a1c", [P, 1], f32, isOutput=False)
    a2bc_p = nc.declare_dram_parameter("a2bc", [P, F], f32, isOutput=False)
    gb4_p = nc.declare_dram_parameter("gb4", [F, 4], f32, isOutput=False)
    out_p = nc.declare_dram_parameter("out", [P, R], f32, isOutput=True)

    RG = [list(range(M_CORES))]

    with tile.TileContext(nc) as tc:
        with (
            tc.tile_pool(name="sb", bufs=1) as sb,
            tc.tile_pool(name="af", bufs=12) as afp,
            tc.tile_pool(name="ep", bufs=4) as epool,
            tc.tile_pool(name="pp", bufs=4) as ppool,
            tc.tile_pool(name="mp", bufs=4) as mpool,
            tc.tile_pool(name="ptp", bufs=8) as ptpool,
            tc.tile_pool(name="pacc", bufs=1, space="PSUM") as pacc,
            tc.tile_pool(name="pcnt", bufs=4, space="PSUM") as pcnt,
            tc.tile_pool(name="poth", bufs=2, space="PSUM") as poth,
            tc.tile_pool(name="dram", bufs=1, space="DRAM") as dram,
        ):
            # ---- dummy collective at t=0: absorbs the one-time cc-stream
            # barrier so the real stats AllReduce at the end is cheap.
            dumb_in = dram.tile([2, 1], f32)
            dumb_sb = sb.tile([2, 1], f32)
            nc.gpsimd.memset(dumb_sb[:], 0.0)
            nc.gpsimd.dma_start(dumb_in[:], dumb_sb[:])
            dumb_out = dram.tile([2, 1], f32, addr_space="Shared")
            nc.gpsimd.collective_compute(
                "AllReduce", Alu.add, replica_groups=RG,
                ins=[dumb_in[:].opt()], outs=[dumb_out[:].opt()])

            # ---- persistent loads ----
            # sync (HWDGE ring 1): adjT first, then the af stream (main loop)
            adjT = sb.tile([P, 16, 2, R], fp8)
            nc.sync.dma_start(adjT[:], adjT_p[:])
            # scalar (HWDGE ring 2): in order of first use by the PE stream
            Wsb = sb.tile([P, 2, 2 * F], f32)
            nc.scalar.dma_start(Wsb[:], Wsb_p[:])
            hTs = sb.tile([P, 4, R], f32)
            nc.scalar.dma_start(hTs[:], hTs_p[:])
            a1c = sb.tile([P, 1], f32)
            nc.scalar.dma_start(a1c[:], a1c_p[:])
            a2bc = sb.tile([P, F], f32)
            nc.scalar.dma_start(a2bc[:], a2bc_p[:])
            gb4 = sb.tile([F, 4], f32)
            nc.scalar.dma_start(gb4[:], gb4_p[:])
            hTfb = sb.tile([P, 4, N], bf16)
            nc.scalar.dma_start(hTfb[:], hTfb_p[:])
            # gpsimd (SWDGE): dinv
            dinv = sb.tile([P, NT, R], fp8)
            nc.gpsimd.dma_start(dinv[:], dinv_p[:])

            ones64 = sb.tile([P, F], f32)
            nc.vector.memset(ones64[:], 1.0)
            ones1 = sb.tile([1, P], f32)
            nc.vector.memset(ones1[:], 1.0)
            # bf16 copy of W for the natural-layout Wh matmuls (bf16 lhsT)
            Wsbb = sb.tile([P, 2, 2 * F], bf16)
            nc.vector.tensor_copy(Wsbb[:], Wsb[:])

            # ---- psum accumulators: [0:64]=numerator, [64:65]=denominator
            acc = [pacc.tile([F + 1, R], f32, tag=f"acc{b}", name=f"acc{b}")
                   for b in range(2)]

            # ---- whf tiles (natural layout, bf16) + ones col + s2 ----
            # whf[:, kt, 0:64]=Wh1, [64:65]=1, [65:129]=Wh2, [129:130]=1
            whf = sb.tile([P, NT, 2 * F + 2], bf16)
            nc.vector.memset(whf[:, :, F:F + 1], 1.0)
            nc.vector.memset(whf[:, :, 2 * F + 1:2 * F + 2], 1.0)
            s2 = sb.tile([P, 2, NT], f32)
            s1bc = sb.tile([P, 2, R], f32)

            def prologue_wh():
                # local Wh^T shard (f32) -> s1 -> s1bc (partition broadcast).
                # a1c rows 0:64 and 64:128 both hold a1 so each branch's
                # matmul reads operands at its own base partition.
                whT_ps = poth.tile([P, R], f32, tag="tmp", name="whTps")
                for b in range(2):
                    for t in range(2):
                        nc.tensor.matmul(
                            whT_ps[F * b:F * (b + 1), :],
                            Wsb[:, t, F * b:F * (b + 1)],
                            hTs[:, 2 * b + t, :],
                            start=(t == 0), stop=(t == 1),
                        )
                whT_sb = sb.tile([P, R], f32, name="whT_sb")
                nc.vector.tensor_copy(whT_sb[:], whT_ps[:])
                for b in range(2):
                    s1_ps = poth.tile([1, R], f32, tag="tmp", name=f"s1ps{b}")
                    nc.tensor.matmul(s1_ps[:], a1c[F * b:F * (b + 1), :],
                                     whT_sb[F * b:F * (b + 1), :],
                                     start=True, stop=True)
                    s1_sb = sb.tile([1, R], f32, tag=f"s1sb{b}",
                                    name=f"s1sb{b}")
                    nc.vector.tensor_copy(s1_sb[:], s1_ps[:])
                    bc_ps = poth.tile([P, R], f32, tag="tmp",
                                      name=f"s1bc{b}")
                    nc.tensor.matmul(bc_ps[:], ones1[:], s1_sb[:],
                                     start=True, stop=True)
                    nc.vector.tensor_copy(s1bc[:, b, :], bc_ps[:])

            def wh_tiles(kt):
                for b in range(2):
                    whn = poth.tile([P, F], f32, tag="tmp",
                                    name=f"whn{kt}_{b}")
                    for t in range(2):
                        nc.tensor.matmul(
                            whn[:],
                            hTfb[:, 2 * b + t, P * kt:P * (kt + 1)],
                            Wsbb[:, t, F * b:F * (b + 1)],
                            start=(t == 0), stop=(t == 1),
                        )
                    scr = epool.tile([P, F], f32, tag="e", name=f"s2s{kt}_{b}")
                    nc.vector.scalar_tensor_tensor(
                        scr[:], whn[:], 1.0, a2bc[:],
                        op0=Alu.mult, op1=Alu.mult,
                        accum_out=s2[:, b, kt:kt + 1])
                    off = (F + 1) * b
                    nc.vector.tensor_copy(whf[:, kt, off:off + F], whn[:])

            def softmax_tile(b, kt, pt_mask_fn):
                e = epool.tile([P, R], f32, tag="e")
                nc.scalar.activation(e[:], s1bc[:, b, :], Act.Prelu,
                                     bias=s2[:, b, kt:kt + 1], alpha=ALPHA)
                p = ppool.tile([P, R], bf16, tag="p")
                nc.scalar.activation(p[:], e[:], Act.Exp)
                pt = pt_mask_fn(p)
                off = (F + 1) * b
                nc.tensor.matmul(acc[b][:], whf[:, kt, off:off + F + 1],
                                 pt[:], start=(kt == 0), stop=(kt == NT - 1))

            def mask_b1(kt):
                def fn(p):
                    pt = ptpool.tile([P, R], bf16, tag="pt")
                    nc.vector.tensor_tensor(pt[:], p[:],
                                            adjT[:, kt // 2, kt % 2, :],
                                            op=Alu.mult)
                    return pt
                return fn

            def mask_b2(kt, cnt):
                def fn(p):
                    m = mpool.tile([P, R], bf16, tag="m")
                    nc.vector.scalar_tensor_tensor(
                        m[:], cnt[:], 1.0, dinv[:, kt, :],
                        op0=Alu.min, op1=Alu.mult)
                    pt = ptpool.tile([P, R], bf16, tag="pt")
                    nc.gpsimd.tensor_tensor(pt[:], p[:], m[:], op=Alu.mult)
                    return pt
                return fn

            # ---- main loop: adj2 DoubleRow groups + interleaved softmax ----
            # PE issue order: g0 DRs | whT/s1 | wh tiles | g1 DRs | att g0 |
            # g2 DRs | att g1 | ... -- att matmuls trail one group behind.
            pend = []          # deferred softmax work: (g, cnts)

            def run_group_softmax(g, cnts):
                for j in range(G):
                    kt = G * g + j
                    softmax_tile(1, kt, mask_b2(kt, cnts[j]))
                    softmax_tile(0, kt, mask_b1(kt))

            for g in range(NG):
                cnts = [pcnt.tile([P, R], f32, tag="cnt", name=f"cnt{g}_{j}")
                        for j in range(G)]
                for tp in range(TP):
                    af = afp.tile([P, 2, 2, R], fp8, tag="af")
                    nc.sync.dma_start(af[:], adjDR_p[g, tp])
                    for j in range(G):
                        for dt in range(2):
                            nc.tensor.matmul(
                                cnts[j][:],
                                af[:, dt, :, P * j:P * (j + 1)],
                                adjT[:, 2 * tp + dt, :, :],
                                perf_mode=DR,
                                start=(tp == 0 and dt == 0),
                                stop=(tp == TP - 1 and dt == 1),
                            )
                if g == 0:
                    prologue_wh()
                    for kt in range(NT):
                        wh_tiles(kt)
                if g in (3, 6):
                    # cc-stream keepalive so the final AllReduce stays warm
                    ka_out = dram.tile([2, 1], f32, addr_space="Shared",
                                       name=f"ka{g}")
                    nc.gpsimd.collective_compute(
                        "AllReduce", Alu.add, replica_groups=RG,
                        ins=[dumb_in[:].opt()], outs=[ka_out[:].opt()])
                pend.append((g, cnts))
                if g >= 1:
                    run_group_softmax(*pend.pop(0))
            while pend:
                run_group_softmax(*pend.pop(0))

            # ---- epilogue: normalize, BN stats AR, BN+lrelu, store ----
            stats_in = dram.tile([2 * F, 2], f32)
            hp = []
            for b in range(2):
                rct = sb.tile([F + 1, R], f32, tag=f"rct{b}")
                nc.vector.reciprocal(rct[F:F + 1, :], acc[b][F:F + 1, :])
                rbc_ps = poth.tile([F, R], f32, tag="tmp", name=f"rbc{b}")
                nc.tensor.matmul(rbc_ps[:], ones64[F:F + 1, :],
                                 rct[F:F + 1, :], start=True, stop=True)
                rbc = sb.tile([F, R], f32, tag=f"rbc{b}")
                nc.vector.tensor_copy(rbc[:], rbc_ps[:])
                hpb = sb.tile([F, R], f32, tag=f"hp{b}")
                nc.vector.tensor_mul(hpb[:], acc[b][0:F, :], rbc[:])
                hp.append(hpb)
                sx = sb.tile([F, 2], f32, tag=f"sx{b}")
                nc.vector.tensor_reduce(sx[:, 0:1], hpb[:],
                                        axis=mybir.AxisListType.X, op=Alu.add)
                scr = ppool.tile([P, R], bf16, tag="p", name=f"sq{b}")
                nc.scalar.activation(scr[0:F, :], hpb[:], Act.Square,
                                     accum_out=sx[:, 1:2])
                nc.gpsimd.dma_start(stats_in[F * b:F * (b + 1), :], sx[:])

            stats_out = dram.tile([2 * F, 2], f32, addr_space="Shared")
            nc.gpsimd.collective_compute(
                "AllReduce", Alu.add, replica_groups=RG,
                ins=[stats_in[:].opt()], outs=[stats_out[:].opt()])

            for b in range(2):
                gst = sb.tile([F, 2], f32, tag=f"gst{b}")
                nc.sync.dma_start(gst[:], stats_out[F * b:F * (b + 1), :])
                mean = sb.tile([F, 1], f32, tag=f"mean{b}")
                nc.scalar.mul(mean[:], gst[:, 0:1], INV_N)
                ex2 = sb.tile([F, 1], f32, tag=f"ex2{b}")
                nc.scalar.mul(ex2[:], gst[:, 1:2], INV_N)
                var = sb.tile([F, 1], f32, tag=f"var{b}")
                nc.vector.scalar_tensor_tensor(var[:], mean[:], -1.0, mean[:],
                                               op0=Alu.mult, op1=Alu.mult)
                nc.vector.tensor_add(var[:], var[:], ex2[:])
                nc.vector.tensor_scalar_add(var[:], var[:], EPS)
                std = sb.tile([F, 1], f32, tag=f"std{b}")
                nc.scalar.activation(std[:], var[:], Act.Sqrt)
                rstd = sb.tile([F, 1], f32, tag=f"rstd{b}")
                nc.vector.reciprocal(rstd[:], std[:])
                scale = sb.tile([F, 1], f32, tag=f"scale{b}")
                nc.vector.tensor_mul(scale[:], gb4[:, 2 * b:2 * b + 1],
                                     rstd[:])
                nbias = sb.tile([F, 1], f32, tag=f"nbias{b}")
                nc.vector.scalar_tensor_tensor(nbias[:], mean[:], -1.0,
                                               scale[:], op0=Alu.mult,
                                               op1=Alu.mult)
                nc.vector.tensor_add(nbias[:], nbias[:],
                                     gb4[:, 2 * b + 1:2 * b + 2])
                # BN affine + final leakyrelu fused in one activation
                fin = sb.tile([F, R], f32, tag=f"fin{b}")
                nc.scalar.activation(fin[:], hp[b][:], Act.Prelu,
                                     bias=nbias[:], scale=scale[:],
                                     alpha=ALPHA)
                nc.gpsimd.dma_start(out_p[F * b:F * (b + 1), :], fin[:])

    nc.compile()
    return nc


def _get_nc():
    if "nc" not in _CACHED:
        _CACHED["nc"] = build_nc()
    return _CACHED["nc"]


def make_in_maps(h, adj, W1, W2, a, gamma, beta):
    fp8 = ml_dtypes.float8_e4m3fn
    bf16 = ml_dtypes.bfloat16
    h = np.asarray(h, dtype=np.float32)
    adj = np.asarray(adj, dtype=np.float32)
    W1 = np.asarray(W1, np.float32)
    W2 = np.asarray(W2, np.float32)
    a_flat = np.asarray(a, np.float32).reshape(2 * F)
    gamma = np.asarray(gamma, np.float32)
    beta = np.asarray(beta, np.float32)

    adj8 = adj.astype(fp8)
    # adjDR[g, tp, p, dt, s, k] = adj[256*(2tp+dt) + 128*s + p, 512*g + k]
    t1 = adj8.reshape(TP, 2, 2, P, NG, R)          # [tp, dt, s, p, g, k]
    adjDR = np.ascontiguousarray(t1.transpose(4, 0, 3, 1, 2, 5))

    hT = h.T                                        # [IN_F, N]
    hTfb = np.ascontiguousarray(
        hT.astype(bf16).reshape(4, P, N).transpose(1, 0, 2))

    Wsb = np.ascontiguousarray(
        np.concatenate([W1, W2], axis=1).reshape(2, P, 2 * F)
        .transpose(1, 0, 2))
    a1c = np.ascontiguousarray(
        np.concatenate([a_flat[:F], a_flat[:F]]).reshape(P, 1))
    a2bc = np.ascontiguousarray(
        np.broadcast_to(a_flat[F:], (P, F)))
    gb4 = np.ascontiguousarray(
        np.stack([gamma[:F], beta[:F], gamma[F:], beta[F:]], axis=1))

    in_maps = []
    for c in range(M_CORES):
        r0 = c * R
        # adjT[p, t, s, i] = adj[r0+i, 256t+128s+p]
        ash = adj8[r0:r0 + R, :].T                  # [N(t_glob), R(i)]
        adjT = np.ascontiguousarray(
            ash.reshape(16, 2, P, R).transpose(2, 0, 1, 3))
        # dinv[p, kt, i] = 0 where 128*kt + p == r0 + i
        dinv = np.ones((P, NT, R), dtype=fp8)
        ii = np.arange(R)
        kk = r0 + ii
        dinv[kk % P, kk // P, ii] = 0
        hTs = np.ascontiguousarray(
            hT[:, r0:r0 + R].reshape(4, P, R).transpose(1, 0, 2))
        in_maps.append({
            "hTs": hTs,
            "hTfb": hTfb,
            "adjT": adjT,
            "adjDR": adjDR,
            "dinv": dinv,
            "Wsb": Wsb,
            "a1c": a1c,
            "a2bc": a2bc,
            "gb4": gb4,
        })
    return in_maps


def kernel(h, adj, W1, W2, a, gamma, beta):
    from concourse.bass_utils import run_bass_kernel_spmd

    in_maps = make_in_maps(h, adj, W1, W2, a, gamma, beta)
    nc = _get_nc()
    res = run_bass_kernel_spmd(nc, in_maps, core_ids=list(range(M_CORES)))
    out = np.empty((N, 2 * F), dtype=np.float32)
    for c in range(M_CORES):
        out[c * R:(c + 1) * R, :] = np.asarray(res.results[c]["out"]).T
    return out
